# revision 28
# baseline (speedup 1.0000x reference)
"""Trainium2 Bass kernel for nn_ContrastMemLoss (SupCon distillation loss).

Self-contained: hardcodes all shapes. Distributes over 8 NeuronCores:
- data-parallel over the batch dim (1 image pair/core) for the BN statistics
  streaming phase, with fp8 DoubleRow matmuls (2x PE throughput) on
  host-quantized e4m3 inputs (W1 prescaled by 64 to dodge e4m3 subnormals;
  folded back in the BN finalize constants),
- 8KB AllReduce of the BN moment vectors,
- anchor embeddings column-sharded 512/core, AllGathered (2x256KiB) into the
  full contrast matrix, row-sharded MxM loss,
- phase-4 loss restructured as sum(mp*(G - ln(exp(G)+neg))) with fused
  scalar_tensor_tensor masked-sum passes; no row-max needed (|G|<=1/TEMP),
- per-core partial loss sums reduced on host.
"""
import sys

if "/opt/trn_rl_repo" not in sys.path:
    sys.path.insert(0, "/opt/trn_rl_repo")

import numpy as np
import ml_dtypes

import bass_rust as _bass_rust
import concourse.bacc as bacc
import concourse.mybir as mybir
import concourse.tile as tile
import concourse.bass_utils as bass_utils
from concourse.hw_specs import get_activation_tables
from concourse.bass import AP  # noqa: F401

F32 = mybir.dt.float32
BF16 = mybir.dt.bfloat16
FP8 = mybir.dt.float8e4

TEMP = 0.07
BASE_TEMP = 0.07
LOSS_WEIGHT = 0.5
BN_EPS = 1e-5
W1_PRESCALE = 64.0

NCORES = 8


class Bacc2(bacc.Bacc):
    """Bacc steering the act-table pass onto natural_log_exp_and_others (holds
    exp+ln+relu+square+copy together) so no ACT_TABLE_LOADs (1.28us each) occur
    mid-kernel. The table LIST ORDER is the act_func_set_id contract with the
    compiler, so we must NOT reorder it; instead strip the functions we use
    from every other set so only the combined set can serve them."""

    PIN = "natural_log_exp_and_others"

    def insert_act_table_loads(self):
        has_activation = any(
            isinstance(i, mybir.InstActivation)
            for b in self.main_func.blocks
            for i in b.instructions
        )
        if not has_activation:
            return
        ours = {
            mybir.ActivationFunctionType.Exp,
            mybir.ActivationFunctionType.Ln,
            mybir.ActivationFunctionType.Square,
            mybir.ActivationFunctionType.Relu,
            mybir.ActivationFunctionType.Copy,
        }
        tables = [
            (name, set(fns) if name == self.PIN else set(fns) - ours)
            for name, fns in get_activation_tables(self.m.arch).items()
        ]
        _bass_rust.insert_act_table_loads(self, tables)


class Dims:
    def __init__(self, C=512, HW=16384, D=256, A=152, V=26, MP=4096, PT=2048):
        self.C = C          # conv channels
        self.HW = HW        # pixels per image
        self.D = D          # feature dim
        self.A = A          # anchors
        self.V = V          # views
        self.M = A * V      # anchor matrix size (unpadded)
        self.MP = MP        # padded
        self.PT = PT        # pixel tile (free dim) for streaming phase
        self.KB = C // 128  # cin blocks (bf16, 128-wide)
        self.KB2 = C // 256  # cin blocks (fp8 DoubleRow, 256-wide)
        self.CB = C // 128  # cout blocks (conv1)
        self.ZB = D // 128  # cout blocks (conv2)
        self.NT = HW // PT  # pixel tiles per image
        self.NCHUNK = PT // 512       # 512-pixel matmul chunks per tile
        self.MC = MP // 512           # anchor column chunks
        self.RPC = MP // NCORES       # rows per core
        self.RB = self.RPC // 128     # row blocks per core
        self.NPIX = NCORES * HW       # total pixels (BN denominator)


def build_kernel(dims: Dims, phases: int = 4, sim_mode: bool = False, nrep: int = 1,
                 dbg: bool = False):
    d = dims
    nc = Bacc2("TRN2", target_bir_lowering=False, debug=False, num_devices=NCORES)
    if dbg:
        dbg_t = nc.dram_tensor("dbg", [128, 2 * d.RB * 3], F32, kind="ExternalOutput")
        dbg_z = nc.dram_tensor("dbgz", [128, 128], F32, kind="ExternalOutput")
        dbg_s = nc.dram_tensor("dbgs", [128, 32], F32, kind="ExternalOutput")

    # ---- per-core DRAM I/O ----
    ximg = nc.dram_tensor("ximg", [2, d.KB2, 128, 2, d.HW], FP8, kind="ExternalInput")
    w1q = nc.dram_tensor("w1q", [2, d.KB2, 128, 2, d.C], FP8, kind="ExternalInput")
    xso = nc.dram_tensor("xso", [2, d.KB, 128, d.RPC], BF16, kind="ExternalInput")
    w1t = nc.dram_tensor("w1t", [2, d.KB, 128, d.C], BF16, kind="ExternalInput")
    w2t = nc.dram_tensor("w2t", [2, d.KB, 128, d.D], BF16, kind="ExternalInput")
    # per-stage channel vectors in [128, CB] layout (c = f*128 + p): gamma, beta, b1
    bnc = nc.dram_tensor("bnc", [2, 3, 128, d.CB], F32, kind="ExternalInput")
    b2c = nc.dram_tensor("b2c", [2, 128, d.ZB], F32, kind="ExternalInput")
    maskp = nc.dram_tensor("maskp", [d.RB, 128, d.MP], BF16, kind="ExternalInput")
    maskn = nc.dram_tensor("maskn", [d.RB, 128, d.MP], BF16, kind="ExternalInput")
    rowco = nc.dram_tensor("rowco", [128, d.RB], F32, kind="ExternalInput")
    pout = nc.dram_tensor("pout", [128, 2 * d.RB], F32, kind="ExternalOutput")

    inv_npix = 1.0 / float(d.NPIX)
    inv_npix_y2 = inv_npix / (W1_PRESCALE * W1_PRESCALE)

    with tile.TileContext(nc) as tc:
        with (
            tc.tile_pool(name="wpool", bufs=1) as wpool,
            tc.tile_pool(name="zpool", bufs=1) as zpool,
            tc.tile_pool(name="mpool", bufs=1) as mpool,
            tc.tile_pool(name="cpool", bufs=1) as cpool,
            tc.tile_pool(name="dram", bufs=1, space="DRAM") as dram,
        ):
            # resident weights
            w1sb = [[wpool.tile([128, d.C], BF16, tag=f"w1_{st}_{kb}", name=f"w1_{st}_{kb}") for kb in range(d.KB)] for st in range(2)]
            w2sb = [[wpool.tile([128, d.D], BF16, tag=f"w2_{st}_{kb}", name=f"w2_{st}_{kb}") for kb in range(d.KB)] for st in range(2)]
            w1qb = [[wpool.tile([128, 2, d.C], FP8, tag=f"w1q_{st}_{kb}", name=f"w1q_{st}_{kb}") for kb in range(d.KB2)] for st in range(2)]
            for st in range(2):
                for kb in range(d.KB):
                    nc.sync.dma_start(w1sb[st][kb][:], w1t[st, kb])
                    nc.sync.dma_start(w2sb[st][kb][:], w2t[st, kb])
                for kb in range(d.KB2):
                    nc.sync.dma_start(w1qb[st][kb][:], w1q[st, kb])
            # resident masks (prefetched at start, used by phase 4)
            mpos = [mpool.tile([128, d.MP], BF16, tag=f"mp{rb}", name=f"mp{rb}") for rb in range(d.RB)]
            mneg = [mpool.tile([128, d.MP], BF16, tag=f"mn{rb}", name=f"mn{rb}") for rb in range(d.RB)]
            for rb in range(d.RB):
                nc.sync.dma_start(mpos[rb][:], maskp[rb])
                nc.sync.dma_start(mneg[rb][:], maskn[rb])
            # resident contrast embeddings (bf16, channel-major) + own-row slices
            ZT = [[zpool.tile([128, d.MP], BF16, tag=f"zt{st}{zb}", name=f"zt{st}{zb}") for zb in range(d.ZB)] for st in range(2)]
            ZO = [[zpool.tile([128, d.RPC], BF16, tag=f"zo{st}{zb}", name=f"zo{st}{zb}") for zb in range(d.ZB)] for st in range(2)]
            # constants
            gsb = [cpool.tile([128, d.CB], F32, tag=f"g{st}", name=f"g{st}") for st in range(2)]
            bsb = [cpool.tile([128, d.CB], F32, tag=f"b{st}", name=f"b{st}") for st in range(2)]
            b1sb = [cpool.tile([128, d.CB], F32, tag=f"b1{st}", name=f"b1{st}") for st in range(2)]
            b2sb = [cpool.tile([128, d.ZB], F32, tag=f"b2{st}", name=f"b2{st}") for st in range(2)]
            for st in range(2):
                nc.sync.dma_start(gsb[st][:], bnc[st, 0])
                nc.sync.dma_start(bsb[st][:], bnc[st, 1])
                nc.sync.dma_start(b1sb[st][:], bnc[st, 2])
                nc.sync.dma_start(b2sb[st][:], b2c[st])
            rcsb = cpool.tile([128, d.RB], F32, tag="rc")
            nc.sync.dma_start(rcsb[:], rowco[:])
            ones_col = cpool.tile([128, 1], BF16, tag="ones_col")
            nc.vector.memset(ones_col[:], 1.0)
            ones_row = cpool.tile([1, 128], BF16, tag="ones_row")
            nc.vector.memset(ones_row[:], 1.0)
            # stats accumulators
            xsum_acc = cpool.tile([128, 8 * d.NT], F32, tag="xsum_acc")
            y2_acc = cpool.tile([128, 2 * d.CB * d.NT], F32, tag="y2_acc")
            xsum_bf = cpool.tile([128, 2 * d.KB], BF16, tag="xsum_bf")
            stat_sb = cpool.tile([128, 16], F32, tag="stat")
            stat2_sb = cpool.tile([128, 16], F32, tag="stat2")
            scale_sb = [cpool.tile([128, d.CB], F32, tag=f"sc{st}", name=f"sc{st}") for st in range(2)]
            shift_sb = [cpool.tile([128, d.CB], F32, tag=f"sh{st}", name=f"sh{st}") for st in range(2)]
            out_sb = cpool.tile([128, 2 * d.RB], F32, tag="out")
            if dbg:
                dbg_sb = cpool.tile([128, 2 * d.RB * 3], F32, tag="dbg_sb", name="dbg_sb")
            else:
                dbg_sb = None
            tmp_a = cpool.tile([128, d.CB], F32, tag="tmp_a")
            tmp_b = cpool.tile([128, d.CB], F32, tag="tmp_b")

            def one_pass():
                # ================= Phase 1: streaming BN moments =================
                ctx_p1 = nc.named_scope("phase1_stats"); ctx_p1.__enter__()
                with (
                    tc.tile_pool(name="xstream", bufs=4) as xpool,
                    tc.tile_pool(name="sqs", bufs=3) as sqspool,
                    tc.tile_pool(name="hbv", bufs=3) as hbpool,
                    tc.tile_pool(name="psum1", bufs=2, space="PSUM") as pp1,
                ):
                    cnt = 0
                    for st in range(2):
                        for t in range(d.NT):
                            xt = []
                            for kb in range(d.KB2):
                                x = xpool.tile([128, 2, d.PT], FP8, tag="x")
                                nc.sync.dma_start(x[:], ximg[st, kb, :, :, t * d.PT:(t + 1) * d.PT])
                                xt.append(x)
                                # xsum col pair: ((st*KB2+kb)*2 .. +2) * NT + t
                                base = ((st * d.KB2 + kb) * 2) * d.NT
                                nc.vector.tensor_reduce(
                                    xsum_acc[:, base + t : base + t + 1],
                                    x[:, 0], axis=mybir.AxisListType.X, op=mybir.AluOpType.add)
                                nc.vector.tensor_reduce(
                                    xsum_acc[:, base + d.NT + t : base + d.NT + t + 1],
                                    x[:, 1], axis=mybir.AxisListType.X, op=mybir.AluOpType.add)
                            for cb in range(d.CB):
                                # 4-bank PSUM tile; chunks are bank-aligned so
                                # per-chunk start zeroing stays within its bank
                                ps = pp1.tile([128, d.PT], F32)
                                for ch in range(d.NCHUNK):
                                    for kb in range(d.KB2):
                                        nc.tensor.matmul(
                                            ps[:, ch * 512:(ch + 1) * 512],
                                            w1qb[st][kb][:, :, cb * 128:(cb + 1) * 128],
                                            xt[kb][:, :, ch * 512:(ch + 1) * 512],
                                            start=(kb == 0), stop=(kb == d.KB2 - 1),
                                            perf_mode=mybir.MatmulPerfMode.DoubleRow)
                                col = (st * d.CB + cb) * d.NT + t
                                cnt += 1
                                if cnt % 4 == 0:
                                    # vector path: PSUM->bf16 copy, then all-SBUF
                                    # fused square+accum (2-PSUM-read rule avoided)
                                    hb = hbpool.tile([128, d.PT], BF16, tag="hb")
                                    nc.vector.tensor_copy(hb[:], ps[:])
                                    sq = hbpool.tile([128, d.PT], BF16, tag="sqv")
                                    with nc.allow_low_precision(reason="bf16 h for E[h^2]; bias ~5e-6"):
                                        nc.vector.scalar_tensor_tensor(
                                            sq[:], hb[:], 1.0, hb[:],
                                            op0=mybir.AluOpType.mult,
                                            op1=mybir.AluOpType.mult,
                                            accum_out=y2_acc[:, col:col + 1])
                                else:
                                    sq = sqspool.tile([128, d.PT], BF16, tag="sqs")
                                    nc.scalar.activation(
                                        sq[:], ps[:], mybir.ActivationFunctionType.Square,
                                        accum_out=y2_acc[:, col:col + 1])

                # ---- finalize per-core moments + matvec mu ----
                with (
                    tc.tile_pool(name="psmv", bufs=2, space="PSUM") as ppmv,
                ):
                    for st in range(2):
                        for kb in range(d.KB):  # kb = kb2*2 + plane
                            base = (st * d.KB + kb) * d.NT
                            col = st * d.KB + kb
                            with nc.allow_low_precision(reason="bf16 Sx for tiny mean matvec"):
                                nc.vector.tensor_reduce(
                                    xsum_bf[:, col:col + 1],
                                    xsum_acc[:, base:base + d.NT], axis=mybir.AxisListType.X,
                                    op=mybir.AluOpType.add)
                    for st in range(2):
                        for cb in range(d.CB):
                            ps = ppmv.tile([128, 1], F32, tag="mv")
                            for kb in range(d.KB):
                                nc.tensor.matmul(
                                    ps[:],
                                    w1sb[st][kb][:, cb * 128:(cb + 1) * 128],
                                    xsum_bf[:, st * d.KB + kb: st * d.KB + kb + 1],
                                    start=(kb == 0), stop=(kb == d.KB - 1))
                            # stat rows: r = 2*st -> mu, r = 2*st+1 -> y2 ; col = r*4 + cb
                            nc.vector.tensor_copy(stat_sb[:, (2 * st) * 4 + cb:(2 * st) * 4 + cb + 1], ps[:])
                            s = (st * d.CB + cb) * d.NT
                            nc.vector.tensor_reduce(
                                stat_sb[:, (2 * st + 1) * 4 + cb:(2 * st + 1) * 4 + cb + 1],
                                y2_acc[:, s:s + d.NT], axis=mybir.AxisListType.X,
                                op=mybir.AluOpType.add)

                ctx_p1.__exit__(None, None, None)
                # ================= Phase 2: AllReduce + BN params =================
                ctx_p2 = nc.named_scope("phase2_allreduce"); ctx_p2.__enter__()
                ar_in = dram.tile([128, 16], F32)
                ar_out = dram.tile([128, 16], F32)
                nc.sync.dma_start(ar_in[:], stat_sb[:])
                if sim_mode:
                    nc.sync.dma_start(ar_out[:], ar_in[:])
                else:
                    nc.gpsimd.collective_compute(
                        "AllReduce", mybir.AluOpType.add,
                        replica_groups=[list(range(NCORES))],
                        ins=[ar_in.opt()], outs=[ar_out.opt()])
                nc.sync.dma_start(stat2_sb[:], ar_out[:])

                for st in range(2):
                    mu = stat2_sb[:, (2 * st) * 4:(2 * st) * 4 + d.CB]
                    y2 = stat2_sb[:, (2 * st + 1) * 4:(2 * st + 1) * 4 + d.CB]
                    # mean = mu/NPIX ; var = y2/(64^2*NPIX) - mean^2
                    nc.vector.tensor_scalar_mul(mu, mu, inv_npix)
                    nc.vector.tensor_scalar_mul(y2, y2, inv_npix_y2)
                    nc.vector.tensor_mul(tmp_a[:], mu, mu)
                    nc.vector.tensor_sub(tmp_a[:], y2, tmp_a[:])
                    # inv_std = exp(-0.5*ln(var+eps))
                    nc.vector.tensor_scalar_add(tmp_a[:], tmp_a[:], BN_EPS)
                    nc.scalar.activation(tmp_b[:], tmp_a[:], mybir.ActivationFunctionType.Ln)
                    nc.scalar.activation(tmp_a[:], tmp_b[:], mybir.ActivationFunctionType.Exp, scale=-0.5)
                    nc.vector.tensor_mul(scale_sb[st][:], gsb[st][:], tmp_a[:])
                    # shift = beta - (mean + b1) * scale
                    nc.vector.tensor_add(tmp_a[:], mu, b1sb[st][:])
                    nc.vector.tensor_mul(tmp_a[:], tmp_a[:], scale_sb[st][:])
                    nc.vector.tensor_sub(shift_sb[st][:], bsb[st][:], tmp_a[:])

                ctx_p2.__exit__(None, None, None)
                # ================= Phase 3: anchor embeddings + AllGather =========
                ctx_p3 = nc.named_scope("phase3_anchors"); ctx_p3.__enter__()
                if phases < 3:
                    nc.sync.dma_start(pout[:], stat2_sb[:, 0:8])
                ag_out = []
                with (
                    tc.tile_pool(name="xa", bufs=4) as xapool,
                    tc.tile_pool(name="hsb", bufs=8) as hpool,
                    tc.tile_pool(name="zsb", bufs=4) as zspool,
                    tc.tile_pool(name="zg", bufs=4) as zgpool,
                    tc.tile_pool(name="nrm", bufs=2) as npool,
                    tc.tile_pool(name="ph", bufs=4, space="PSUM") as pph,
                    tc.tile_pool(name="pz", bufs=2, space="PSUM") as ppz,
                    tc.tile_pool(name="pn", bufs=1, space="PSUM") as ppn,
                    tc.tile_pool(name="pb", bufs=1, space="PSUM") as ppb,
                ):
                    for st in range(2 if phases >= 3 else 0):
                        width = d.RPC  # 512: own columns only
                        xa = []
                        for kb in range(d.KB):
                            x = xapool.tile([128, width], BF16, tag="xa")
                            nc.sync.dma_start(x[:], xso[st, kb])
                            xa.append(x)
                        hs = []
                        for cb in range(d.CB):
                            ph = pph.tile([128, width], F32)
                            for kb in range(d.KB):
                                nc.tensor.matmul(
                                    ph[:],
                                    w1sb[st][kb][:, cb * 128:(cb + 1) * 128],
                                    xa[kb][:],
                                    start=(kb == 0), stop=(kb == d.KB - 1))
                            h = hpool.tile([128, width], BF16, tag="h")
                            nc.scalar.activation(
                                h[:], ph[:], mybir.ActivationFunctionType.Relu,
                                bias=shift_sb[st][:, cb:cb + 1], scale=scale_sb[st][:, cb:cb + 1])
                            hs.append(h)
                        zs = []
                        zqs = []
                        pn = ppn.tile([1, width], F32, tag="pn")
                        for zb in range(d.ZB):
                            pz = ppz.tile([128, width], F32)
                            for cb in range(d.CB):
                                nc.tensor.matmul(
                                    pz[:],
                                    w2sb[st][cb][:, zb * 128:(zb + 1) * 128],
                                    hs[cb][:],
                                    start=(cb == 0), stop=(cb == d.CB - 1))
                            z = zspool.tile([128, width], F32, tag="z")
                            nc.vector.tensor_scalar_add(z[:], pz[:], b2sb[st][:, zb:zb + 1])
                            zs.append(z)
                            zq = hpool.tile([128, width], BF16, tag="zq")
                            nc.scalar.activation(zq[:], z[:], mybir.ActivationFunctionType.Square)
                            zqs.append(zq)
                        for zb in range(d.ZB):
                            nc.tensor.matmul(pn[:], ones_col[:], zqs[zb][:],
                                             start=(zb == 0), stop=(zb == d.ZB - 1))
                        # invn = exp(-0.5*ln(max(ssq,1e-24))) broadcast to 128 partitions
                        nm = npool.tile([1, width], F32, tag="nm")
                        nc.vector.tensor_scalar_max(nm[:], pn[:], 1e-24)
                        nc.scalar.activation(nm[:], nm[:], mybir.ActivationFunctionType.Ln)
                        nmb = npool.tile([1, width], BF16, tag="nmb")
                        nc.scalar.activation(nmb[:], nm[:], mybir.ActivationFunctionType.Exp, scale=-0.5)
                        pb = ppb.tile([128, width], F32, tag="pb")
                        nc.tensor.matmul(pb[:], ones_row[:], nmb[:], start=True, stop=True)
                        zg = []
                        for zb in range(d.ZB):
                            g = zgpool.tile([128, width], BF16, tag="zg")
                            nc.vector.tensor_mul(g[:], zs[zb][:], pb[:])
                            zg.append(g)
                            # own rows, scaled by 1/TEMP (folds the logit scale)
                            nc.vector.tensor_scalar_mul(ZO[st][zb][:], g[:], 1.0 / TEMP)
                        # AllGather this stage's embeddings
                        agi = dram.tile([d.ZB, 128, width], BF16, name=f"agi{st}", tag=f"agi{st}")
                        ago = dram.tile([NCORES, d.ZB, 128, width], BF16, name=f"ago{st}", tag=f"ago{st}", addr_space="Shared")
                        for zb in range(d.ZB):
                            nc.sync.dma_start(agi[zb], zg[zb][:])
                        if sim_mode:
                            nc.sync.dma_start(ago[0], agi[:])
                        else:
                            nc.gpsimd.collective_compute(
                                "AllGather", mybir.AluOpType.bypass,
                                replica_groups=[list(range(NCORES))],
                                ins=[agi.opt()], outs=[ago.opt()])
                        ag_out.append(ago)
                    # load gathered contrast matrices
                    for st in range(2 if phases >= 4 else 0):
                        for c in range(NCORES):
                            for zb in range(d.ZB):
                                nc.sync.dma_start(
                                    ZT[st][zb][:, c * d.RPC:(c + 1) * d.RPC],
                                    ag_out[st][c, zb])

                ctx_p3.__exit__(None, None, None)
                # ================= Phase 4: logits + SupCon loss =================
                ctx_p4 = nc.named_scope("phase4_loss"); ctx_p4.__enter__()
                NW = d.MP // d.PT  # wide 4-bank chunks per row block (2)
                with (
                    tc.tile_pool(name="esb", bufs=3) as epool,
                    tc.tile_pool(name="ldsb", bufs=2) as ldpool,
                    tc.tile_pool(name="scr", bufs=2) as scrpool,
                    tc.tile_pool(name="col", bufs=8) as colpool,
                    tc.tile_pool(name="pg", bufs=2, space="PSUM") as ppg,
                ):
                    # dir an=1 first: needs ZT[0] (first AllGather); overlaps AG(1)
                    iters = [(an, co, rb)
                             for an, co in (((1, 0), (0, 1)) if phases >= 4 else ())
                             for rb in range(d.RB)]

                    def emit_head(an, co, rb):
                        e = epool.tile([128, d.MP], BF16, tag="e")
                        scr = scrpool.tile([128, d.MP], BF16, tag="scr")
                        s2c = colpool.tile([128, NW], F32, tag="s2c")
                        for w in range(NW):
                            pg = ppg.tile([128, d.PT], F32)
                            for ch in range(d.PT // 512):
                                cc = w * (d.PT // 512) + ch
                                for zb in range(d.ZB):
                                    nc.tensor.matmul(
                                        pg[:, ch * 512:(ch + 1) * 512],
                                        ZO[an][zb][:, rb * 128:(rb + 1) * 128],
                                        ZT[co][zb][:, cc * 512:(cc + 1) * 512],
                                        start=(zb == 0), stop=(zb == d.ZB - 1))
                            # e = exp(G) (no row-max: |G| <= 1/TEMP = 14.3)
                            nc.scalar.activation(
                                e[:, w * d.PT:(w + 1) * d.PT], pg[:],
                                mybir.ActivationFunctionType.Exp)
                            # s2 partial: sum(mp * G) straight from PSUM
                            nc.vector.scalar_tensor_tensor(
                                scr[:, w * d.PT:(w + 1) * d.PT], pg[:], 1.0,
                                mpos[rb][:, w * d.PT:(w + 1) * d.PT],
                                op0=mybir.AluOpType.mult, op1=mybir.AluOpType.mult,
                                accum_out=s2c[:, w:w + 1])
                        return e, scr, s2c

                    def emit_tail(an, co, rb, e, scr, s2c):
                        # neg = sum(mn * e)
                        negr = colpool.tile([128, 1], F32, tag="negr")
                        nc.vector.scalar_tensor_tensor(
                            scr[:], e[:], 1.0, mneg[rb][:],
                            op0=mybir.AluOpType.mult, op1=mybir.AluOpType.mult,
                            accum_out=negr[:])
                        # lnv = ln(e + neg)
                        ld = ldpool.tile([128, d.MP], BF16, tag="ld")
                        nc.scalar.activation(ld[:], e[:], mybir.ActivationFunctionType.Ln,
                                             bias=negr[:])
                        # s1 = sum(mp * lnv)
                        s1r = colpool.tile([128, 1], F32, tag="s1r")
                        gscr = scrpool.tile([128, d.MP], BF16, tag="gscr")
                        nc.vector.scalar_tensor_tensor(
                            gscr[:], ld[:], 1.0, mpos[rb][:],
                            op0=mybir.AluOpType.mult, op1=mybir.AluOpType.mult,
                            accum_out=s1r[:])
                        s2r = colpool.tile([128, 1], F32, tag="s2r")
                        nc.vector.tensor_reduce(s2r[:], s2c[:], axis=mybir.AxisListType.X,
                                                op=mybir.AluOpType.add)
                        # out = (s2 - s1) * rowco
                        sd = colpool.tile([128, 1], F32, tag="sd")
                        nc.vector.tensor_sub(sd[:], s2r[:], s1r[:])
                        nc.vector.tensor_mul(
                            out_sb[:, an * d.RB + rb:an * d.RB + rb + 1],
                            sd[:], rcsb[:, rb:rb + 1])
                        if dbg:
                            k = (an * d.RB + rb) * 3
                            nc.vector.tensor_copy(dbg_sb[:, k:k+1], negr[:])
                            nc.vector.tensor_copy(dbg_sb[:, k+1:k+2], s1r[:])
                            nc.vector.tensor_copy(dbg_sb[:, k+2:k+3], s2r[:])

                    # 1-deep software pipeline: emit iter k's head, then iter
                    # k-1's tail, so exp(k) hides the neg->ln->s1 chain of k-1.
                    pend = None
                    for it in iters:
                        head = emit_head(*it)
                        if pend is not None:
                            emit_tail(*pend[0], *pend[1])
                        pend = (it, head)
                    if pend is not None:
                        emit_tail(*pend[0], *pend[1])
                    if phases >= 4:
                        nc.sync.dma_start(pout[:], out_sb[:])
                        if dbg:
                            nc.sync.dma_start(dbg_t[:], dbg_sb[:])
                            ds_ = scrpool.tile([128, 32], F32, tag="ds_", name="ds_")
                            for st_ in range(2):
                                nc.vector.tensor_copy(ds_[:, st_*8:st_*8+4], scale_sb[st_][:])
                                nc.vector.tensor_copy(ds_[:, st_*8+4:st_*8+8], shift_sb[st_][:])
                            nc.vector.tensor_copy(ds_[:, 16:32], stat2_sb[:])
                            nc.sync.dma_start(dbg_s[:], ds_[:])
                            dz = scrpool.tile([128, 128], F32, tag="dz")
                            nc.vector.tensor_copy(dz[:, 0:32], ZO[0][0][:, 0:32])
                            nc.vector.tensor_copy(dz[:, 32:64], ZO[1][0][:, 0:32])
                            nc.vector.tensor_copy(dz[:, 64:96], ZT[0][0][:, 512:544])
                            nc.vector.tensor_copy(dz[:, 96:128], ZT[1][0][:, 512:544])
                            nc.sync.dma_start(dbg_z[:], dz[:])
                    elif phases == 3:
                        for st in range(2):
                            for zb in range(d.ZB):
                                nc.vector.tensor_copy(out_sb[:, (st*d.ZB+zb):(st*d.ZB+zb)+1], ZO[st][zb][:, 0:1])
                        nc.sync.dma_start(pout[:], out_sb[:])
                ctx_p4.__exit__(None, None, None)

            for _ in range(nrep):
                one_pass()

    nc.compile()
    return nc


def prep_inputs(inputs, dims: Dims):
    """Host-side sharding/prep. Returns in_maps (one dict per core)."""
    d = dims
    f32 = np.float32
    bf16 = ml_dtypes.bfloat16
    fp8 = ml_dtypes.float8_e4m3
    preds = {0: inputs["preds_S"], 1: inputs["preds_T"]}
    sb = np.asarray(inputs["sample_batch"]).astype(np.int64)
    si = np.asarray(inputs["sample_idx"]).astype(np.int64)
    labels = np.asarray(inputs["labels_"])
    N = preds[0].shape[0]

    m_idx = np.arange(d.M)
    a_of_m = m_idx % d.A
    v_of_m = m_idx // d.A
    b_arr = sb[a_of_m]
    p_arr = si[a_of_m, v_of_m]

    W1 = {st: np.asarray(inputs[f"{p}_W1"]).astype(f32) for st, p in ((0, "s"), (1, "t"))}
    W2 = {st: np.asarray(inputs[f"{p}_W2"]).astype(f32) for st, p in ((0, "s"), (1, "t"))}

    # anchor pixels, channel-major, padded [2, C, MP] -> per-core own columns
    xst_np = np.zeros((2, d.C, d.MP), dtype=bf16)
    for st in range(2):
        X = np.asarray(preds[st]).reshape(N, d.C, d.HW)
        xs = X[b_arr, :, p_arr].astype(f32)  # [M, C]
        xst_np[st, :, :d.M] = xs.T.astype(bf16)
    xst_b = xst_np.reshape(2, d.KB, 128, d.MP)

    # bf16 weights [2, KB, 128, C/D]
    w1t_b = np.stack([W1[st].T.reshape(d.KB, 128, d.C) for st in range(2)]).astype(bf16)
    w2t_b = np.stack([W2[st].T.reshape(d.KB, 128, d.D) for st in range(2)]).astype(bf16)
    # fp8 W1 (prescaled): [2, KB2, 128, 2, C]; channel c = kb2*256 + i*128 + p
    w1q_b = np.stack([
        (W1[st].T * W1_PRESCALE).reshape(d.KB2, 2, 128, d.C).transpose(0, 2, 1, 3)
        for st in range(2)
    ]).astype(fp8)

    bnc_np = np.zeros((2, 3, 128, d.CB), f32)
    b2c_np = np.zeros((2, 128, d.ZB), f32)
    for st, p in ((0, "s"), (1, "t")):
        for j, nm in enumerate(("gamma", "beta", "b1")):
            bnc_np[st, j] = np.asarray(inputs[f"{p}_{nm}"]).astype(f32).reshape(d.CB, 128).T
        b2c_np[st] = np.asarray(inputs[f"{p}_b2"]).astype(f32).reshape(d.ZB, 128).T

    # masks
    base = (labels[:, None] == labels[None, :]).astype(f32)
    mask_full = np.tile(base, (d.V, d.V))
    mask_pos = mask_full * (1.0 - np.eye(d.M, dtype=f32))
    maskp_np = np.zeros((d.MP, d.MP), dtype=bf16)
    maskn_np = np.zeros((d.MP, d.MP), dtype=bf16)
    maskp_np[:d.M, :d.M] = mask_pos
    maskn_np[:d.M, :d.M] = (1.0 - mask_full)
    row_coef = np.zeros(d.MP, f32)
    row_coef[:d.M] = -LOSS_WEIGHT * (TEMP / BASE_TEMP) / d.M / (mask_pos.sum(axis=1) + 1e-6)

    in_maps = []
    for c in range(NCORES):
        m = {}
        img = np.stack([np.asarray(preds[st]).reshape(N, d.C, d.HW)[c % N] for st in range(2)])
        # fp8 image: [2, KB2, 128, 2, HW]
        m["ximg"] = np.ascontiguousarray(
            img.reshape(2, d.KB2, 2, 128, d.HW).transpose(0, 1, 3, 2, 4)).astype(fp8)
        r0, r1 = c * d.RPC, (c + 1) * d.RPC
        m["xso"] = np.ascontiguousarray(xst_b[:, :, :, r0:r1])
        m["w1t"] = w1t_b
        m["w2t"] = w2t_b
        m["w1q"] = w1q_b
        m["bnc"] = bnc_np
        m["b2c"] = b2c_np
        m["maskp"] = np.ascontiguousarray(maskp_np[r0:r1].reshape(d.RB, 128, d.MP))
        m["maskn"] = np.ascontiguousarray(maskn_np[r0:r1].reshape(d.RB, 128, d.MP))
        m["rowco"] = np.ascontiguousarray(row_coef[r0:r1].reshape(d.RB, 128).T)
        in_maps.append(m)
    return in_maps


_CACHED = {}


def kernel(**inputs):
    dims = Dims()
    if "nc" not in _CACHED:
        _CACHED["nc"] = build_kernel(dims)
    nc = _CACHED["nc"]
    in_maps = prep_inputs(inputs, dims)
    res = bass_utils.run_bass_kernel_spmd(nc, in_maps, core_ids=list(range(NCORES)))
    total = np.float64(0.0)
    for r in res.results:
        total += np.float64(r["pout"].sum(dtype=np.float64))
    return np.float32(total)


if __name__ == "__main__":
    d = np.load("/root/problem/work/inputs.npz")
    inputs = {k: d[k] for k in d.files}
    expected = np.load("/root/problem/work/expected.npy")
    out = kernel(**inputs)
    print("expected:", expected, "actual:", out, "rel:", abs(out - expected) / abs(expected))
